# revision 31
# baseline (speedup 1.0000x reference)
"""LayerNorm-LSTM cell (nn_LSTMCell) Trainium2 Bass kernel.

Strategy: data-parallel over the batch dim — each of the 8 NeuronCores
processes 1024 of the 8192 batch rows with replicated weights.

The gate matmul (the whole cost: [1024, 2048] @ [2048, 4096] per core) runs
in fp8-e4m3 with MatmulPerfMode.DoubleRow (2 fp8 weights packed per PE
cell, K=256 per instruction, 0.5 cycles per output column).  Plain fp8 is
too noisy for the 2e-2 gate (measured 2.7e-2), so operands are split into
hi+lo e4m3 parts at one global power-of-2 scale (lo = e4m3 of the residual,
same scale, so all partial products accumulate in the same PSUM group):

  gates ~= xh_hi@W_hi  +  xh_lo@W_hi (NA/8 of K)  +  xh_hi@W_lo (NW/8 of K)

NA/NW in [0..8] trade accuracy for PE time (each unit is ~6.8us).  The
pre-LN gates feed only a per-row group layernorm, which is scale-invariant,
so x/h/W are all scaled by 32 (exact power of two) to keep e4m3 operands
out of the subnormal range; only the LN epsilon must be rescaled by the
gate variance factor (32*32)^2.

Per-core pipeline (B=1024 rows, I=H=1024, 4H=4096):
  gates accumulation per (gate, 128-row block, 512-col half) in PSUM,
  term-major over rounds of 4 blocks (8 PSUM banks) so the startup DMA
  (xh + gate-0 weights) overlaps the first matmuls; per-gate groupnorm via
  bn_stats on PSUM + fused scale/bias activation on ScalarE; LSTM cell
  updates on VectorE; second LN + tanh; stores via gpsimd DMA.
"""

import sys

if "/opt/trn_rl_repo" not in sys.path:
    sys.path.insert(0, "/opt/trn_rl_repo")

import ml_dtypes
import numpy as np

import concourse.bass as bass
import concourse.mybir as mybir
import concourse.tile as tile
from concourse.bass_utils import run_bass_kernel_spmd

P = 128
B, I, H = 8192, 1024, 1024
G4 = 4 * H
K2 = 2 * I                # concat contraction dim (x then h)
KS2 = K2 // P             # 16 k-subtiles of 128
NCORES = 8
BC = B // NCORES          # 1024 batch rows per core
NB = BC // P              # 8 row blocks per core
EPS = 1e-3
FORGET_BIAS = 1.0
BF16 = mybir.dt.bfloat16
F32 = mybir.dt.float32
FP8 = mybir.dt.float8e4
DR = mybir.MatmulPerfMode.DoubleRow
AF = mybir.ActivationFunctionType

SCALE = 32.0              # power-of-2 operand scale (cancels in the LN)
VAR_SCALE = (SCALE * SCALE) ** 2

# correction depth: k-pairs (of 8) getting activation-/weight-residual terms
NA = 8
NW = 8

# ---------------------------------------------------------------------------
# Workaround: the walrus build in this container rejects TPB CTRL
# instructions carrying more than ONE semaphore wait ("Too many sync wait
# commands").  Split fat wait lists into single-wait NoOps on the same
# engine, inserted immediately before the instruction (semantics identical:
# all waits must hold before the instruction executes either way).
_TPB_ENGINES = None


def _split_fat_waits(nc, max_waits=1):
    global _TPB_ENGINES
    if _TPB_ENGINES is None:
        _TPB_ENGINES = {
            mybir.EngineType.PE,
            mybir.EngineType.Activation,
            mybir.EngineType.DVE,
            mybir.EngineType.Pool,
            mybir.EngineType.SP,
        }
    n = 0
    for func in nc.m.functions:
        for bb in func.blocks:
            out = []
            for ins in bb.instructions:
                si = getattr(ins, "sync_info", None)
                eng = getattr(ins, "engine", None)
                if (
                    si is not None
                    and si.on_wait
                    and len(si.on_wait) > max_waits
                    and eng in _TPB_ENGINES
                ):
                    waits = list(si.on_wait)
                    overflow, keep = waits[:-max_waits], waits[-max_waits:]
                    for cs in range(0, len(overflow), max_waits):
                        nop = mybir.InstNoOp(
                            name=f"{ins.name}-ws{cs}",
                            engine=eng,
                            sync_info=mybir.SyncInfo(
                                on_wait=overflow[cs : cs + max_waits], on_update=[]
                            ),
                            text_hint="waitsplit",
                        )
                        out.append(nop)
                        n += 1
                    si.on_wait = keep
                out.append(ins)
            bb.instructions = out
    return n


# ---------------------------------------------------------------------------


def _build(trivial):
    """Build the per-core Bass program.  `trivial` skips the (identity)
    groupnorm affine and the (zero) pre-norm bias."""
    nc = bass.Bass("TRN2", target_bir_lowering=False, debug=False, num_devices=NCORES)

    xh_hi = nc.declare_dram_parameter("xh_hi", [K2, BC], FP8, isOutput=False).ap()
    if NA:
        xh_lo = nc.declare_dram_parameter("xh_lo", [K2, BC], FP8, isOutput=False).ap()
    c_in = nc.declare_dram_parameter("c", [BC, H], BF16, isOutput=False).ap()
    w_hi = nc.declare_dram_parameter("w_hi", [K2, G4], FP8, isOutput=False).ap()
    if NW:
        w_lo = nc.declare_dram_parameter("w_lo", [K2, G4], FP8, isOutput=False).ap()
    if not trivial:
        biasv = nc.declare_dram_parameter("biasv", [1, G4], BF16, isOutput=False).ap()
        g4v = nc.declare_dram_parameter("g4v", [1, G4], F32, isOutput=False).ap()
        b4v = nc.declare_dram_parameter("b4v", [1, G4], F32, isOutput=False).ap()
        gcv = nc.declare_dram_parameter("gcv", [1, H], F32, isOutput=False).ap()
        bcv = nc.declare_dram_parameter("bcv", [1, H], F32, isOutput=False).ap()
    new_h = nc.declare_dram_parameter("new_h", [BC, H], BF16, isOutput=True).ap()
    new_c = nc.declare_dram_parameter("new_c", [BC, H], BF16, isOutput=True).ap()

    xh_hi_r = xh_hi.rearrange("(ks p) b -> p ks b", p=P)
    if NA:
        xh_lo_r = xh_lo.rearrange("(ks p) b -> p ks b", p=P)
    w_hi_r = w_hi.rearrange("(ks p) n -> p ks n", p=P)
    if NW:
        w_lo_r = w_lo.rearrange("(ks p) n -> p ks n", p=P)

    with tile.TileContext(nc) as tc:
        with (
            tc.tile_pool(name="resxh", bufs=1) as resxh,
            tc.tile_pool(name="wphi", bufs=2) as wphi,
            tc.tile_pool(name="wplo", bufs=2) as wplo,
            tc.tile_pool(name="psum0", bufs=2, space="PSUM") as psump0,
            tc.tile_pool(name="psum1", bufs=2, space="PSUM") as psump1,
            tc.tile_pool(name="psum2", bufs=2, space="PSUM") as psump2,
            tc.tile_pool(name="psum3", bufs=2, space="PSUM") as psump3,
            tc.tile_pool(name="acti", bufs=14) as actip,
            tc.tile_pool(name="cp", bufs=2) as cp,
            tc.tile_pool(name="ncp", bufs=3) as ncp,
            tc.tile_pool(name="nhp", bufs=3) as nhp,
            tc.tile_pool(name="stat", bufs=10) as statp,
            tc.tile_pool(name="small", bufs=24) as smallp,
            tc.tile_pool(name="singles", bufs=1) as singles,
            tc.tile_pool(name="gen", bufs=4) as genp,
        ):
            _PSUMPS = (psump0, psump1, psump2, psump3)
            eps1_t = singles.tile([P, 1], F32)
            nc.vector.memset(eps1_t, EPS * VAR_SCALE)
            eps2_t = singles.tile([P, 1], F32)
            nc.vector.memset(eps2_t, EPS)
            one1_t = singles.tile([P, 1], F32)
            nc.vector.memset(one1_t, 1.0)

            if not trivial:
                ones_t = singles.tile([1, P], BF16)
                nc.vector.memset(ones_t, 1.0)
                bias_sb = singles.tile([1, G4], BF16)
                nc.sync.dma_start(out=bias_sb, in_=biasv[:])
                # replicate gamma/beta across all 128 partitions via DMA
                g4_sb = singles.tile([P, G4], F32)
                b4_sb = singles.tile([P, G4], F32)
                gc_sb = singles.tile([P, H], F32)
                bc_sb = singles.tile([P, H], F32)
                for vec, sb, width in (
                    (g4v, g4_sb, G4),
                    (b4v, b4_sb, G4),
                    (gcv, gc_sb, H),
                    (bcv, bc_sb, H),
                ):
                    bcast = bass.AP(
                        tensor=vec.tensor,
                        offset=vec.offset,
                        ap=[[0, P], vec.ap[1]],
                    )
                    nc.sync.dma_start(out=sb, in_=bcast)

            # resident fp8 activations [128, 16, 1024]
            xh_hi_sb = resxh.tile([P, KS2, BC], FP8)
            xh_lo_sb = None
            if NA:
                xh_lo_sb = resxh.tile([P, 2 * NA, BC], FP8, name="xh_lo_sb")

            def load_w(g):
                # HWDGE charges a fixed 625ns per DMA instruction (globally
                # serialized), so batch chunks per instruction.  Gate 0 loads
                # in kp-granular "columns" (one k-pair of every tensor, in
                # term order) so the PE can track DMA arrival exactly;
                # later gates are prefetched in 4-subtile batches.
                gc0 = g * H
                whi = wphi.tile([P, KS2, H], FP8, tag="whi")
                wlo = None
                if NW:
                    wlo = wplo.tile([P, 2 * NW, H], FP8, tag="wlo", name="wlo")
                if g == 0:
                    for kp in range(KS2 // 2):
                        k2 = 2 * kp
                        nc.sync.dma_start(
                            out=whi[:, k2 : k2 + 2, :], in_=w_hi_r[:, k2 : k2 + 2, gc0 : gc0 + H]
                        )
                        nc.sync.dma_start(
                            out=xh_hi_sb[:, k2 : k2 + 2, :], in_=xh_hi_r[:, k2 : k2 + 2, :]
                        )
                        if kp < NA:
                            nc.sync.dma_start(
                                out=xh_lo_sb[:, k2 : k2 + 2, :], in_=xh_lo_r[:, k2 : k2 + 2, :]
                            )
                        if kp < NW:
                            nc.sync.dma_start(
                                out=wlo[:, k2 : k2 + 2, :], in_=w_lo_r[:, k2 : k2 + 2, gc0 : gc0 + H]
                            )
                else:
                    for ks in range(0, KS2, 4):
                        nc.sync.dma_start(
                            out=whi[:, ks : ks + 4, :], in_=w_hi_r[:, ks : ks + 4, gc0 : gc0 + H]
                        )
                    for ks in range(0, 2 * NW, 4):
                        ke = min(ks + 4, 2 * NW)
                        nc.sync.dma_start(
                            out=wlo[:, ks:ke, :], in_=w_lo_r[:, ks:ke, gc0 : gc0 + H]
                        )
                return whi, wlo

            m1s = [None] * NB     # sig(i)*tanh(j), bf16 per block
            tclns = [None] * NB   # tanh(LN(new_c)), bf16 per block
            cbs = [None] * NB

            def stats_rstd_negmu(ps_pair, add_forget, eps_t):
                """bn stats over the two 512-wide halves -> (rstd, bias) APs.

                The small scalar chain runs on the (otherwise idle) Pool
                engine: the DVE otherwise convoys on 0.66us bn_stats of later
                blocks, delaying the PSUM-freeing activations by ~3us."""
                st = statp.tile([P, 2, 6], F32)
                nc.vector.bn_stats(out=st[:, 0, :], in_=ps_pair[0])
                nc.vector.bn_stats(out=st[:, 1, :], in_=ps_pair[1])
                mv = statp.tile([P, 2], F32)
                nc.vector.bn_aggr(out=mv, in_=st)
                mean, var = mv[:, 0:1], mv[:, 1:2]
                sd = smallp.tile([P, 1], F32)
                nc.scalar.activation(sd, var, AF.Sqrt, bias=eps_t, scale=1.0)
                rs = smallp.tile([P, 1], F32)
                nc.vector.reciprocal(rs, sd)
                nm = smallp.tile([P, 1], F32)
                nc.vector.tensor_mul(nm, mean, rs)
                if add_forget:
                    # bias = 1 - mean*rstd
                    nc.vector.tensor_scalar(
                        out=nm, in0=nm, scalar1=-1.0, scalar2=FORGET_BIAS,
                        op0=mybir.AluOpType.mult, op1=mybir.AluOpType.add,
                    )
                else:
                    nc.vector.tensor_scalar_mul(out=nm, in0=nm, scalar1=-1.0)
                return rs, nm

            # --- software-pipelined consumption ---------------------------
            # chain_a(b): stats -> rs/nm, issued right at group close.
            # chain_b(b): activations + cell updates, deferred one round so
            #   no engine's in-order queue head-blocks on a fresh stats chain
            #   (sqrt(b) would otherwise sit behind acts(b-1), which wait on
            #   nm(b-1), serializing all block chains end-to-end).
            # chain_c(b): g2's tanh(LN(new_c)), deferred one more round.
            pendingA = []
            pendingB = []

            def chain_a(g, b, ps_pair):
                rs, nm = stats_rstd_negmu(
                    ps_pair, add_forget=(trivial and g == 2), eps_t=eps1_t
                )
                if g == 0 and b % 4 == 0:
                    # prefetch c, 4 blocks per DMA (used in gate-f phase)
                    cb4 = cp.tile([P, 4, H], BF16, tag="c", name="cb4")
                    nc.sync.dma_start(
                        out=cb4,
                        in_=c_in.rearrange("(nb p) h -> p nb h", p=P)[:, b : b + 4, :],
                    )
                    for bb in range(4):
                        cbs[b + bb] = cb4[:, bb, :]
                pendingA.append((g, b, ps_pair, rs, nm))

            def chain_b(g, b, ps_pair, rs, nm):
                b0 = b * P
                gc0 = g * H
                func = AF.Tanh if g == 1 else AF.Sigmoid
                act = actip.tile([P, H], BF16, tag="act", name="act")
                for half in range(2):
                    hc = half * 512
                    if trivial:
                        nc.scalar.activation(
                            act[:, hc : hc + 512], ps_pair[half], func,
                            bias=nm, scale=rs,
                        )
                    else:
                        t2 = genp.tile([P, 512], F32, tag="gtmp", name="t2")
                        # (x*r) + (-mu*r) == (x-mu)*r
                        nc.vector.tensor_scalar(
                            out=t2, in0=ps_pair[half],
                            scalar1=rs, scalar2=nm,
                            op0=mybir.AluOpType.mult, op1=mybir.AluOpType.add,
                        )
                        nc.vector.tensor_mul(
                            t2, t2, g4_sb[:, gc0 + hc : gc0 + hc + 512]
                        )
                        nc.vector.tensor_add(
                            t2, t2, b4_sb[:, gc0 + hc : gc0 + hc + 512]
                        )
                        nc.scalar.activation(
                            act[:, hc : hc + 512], t2, func,
                            bias=(FORGET_BIAS if g == 2 else 0.0), scale=1.0,
                        )

                if g == 0:
                    m1s[b] = act
                elif g == 1:
                    # m1 = sig(i) * tanh(j), in place over sig(i)
                    nc.vector.tensor_mul(m1s[b], m1s[b], act)
                elif g == 2:
                    ncv = ncp.tile([P, H], BF16, tag="nc", name="ncv")
                    nc.vector.tensor_mul(ncv, cbs[b], act)
                    nc.vector.tensor_add(ncv, ncv, m1s[b])
                    nc.sync.dma_start(out=new_c[b0 : b0 + P, :], in_=ncv)
                    # LN stats over new_c (tanh deferred to chain_c)
                    st2 = statp.tile([P, 2, 6], F32, name="st2")
                    nc.vector.bn_stats(out=st2[:, 0, :], in_=ncv[:, 0:512])
                    nc.vector.bn_stats(out=st2[:, 1, :], in_=ncv[:, 512:1024])
                    mv2 = statp.tile([P, 2], F32, name="mv2")
                    nc.vector.bn_aggr(out=mv2, in_=st2)
                    sd2 = smallp.tile([P, 1], F32, name="sd2")
                    nc.scalar.activation(
                        sd2, mv2[:, 1:2], AF.Sqrt, bias=eps2_t, scale=1.0
                    )
                    rs2 = smallp.tile([P, 1], F32, name="rs2")
                    nc.vector.reciprocal(rs2, sd2)
                    nm2 = smallp.tile([P, 1], F32, name="nm2")
                    nc.vector.tensor_mul(nm2, mv2[:, 0:1], rs2)
                    nc.vector.tensor_scalar_mul(out=nm2, in0=nm2, scalar1=-1.0)
                    pendingB.append((b, ncv, rs2, nm2))
                else:
                    nh = nhp.tile([P, H], BF16, tag="nh", name="nh")
                    nc.vector.tensor_mul(nh, tclns[b], act)
                    nc.sync.dma_start(out=new_h[b0 : b0 + P, :], in_=nh)

            def chain_c(b, ncv, rs2, nm2):
                tcl = actip.tile([P, H], BF16, tag="act", name="tcl")
                if trivial:
                    nc.scalar.activation(tcl, ncv, AF.Tanh, bias=nm2, scale=rs2)
                else:
                    t3 = genp.tile([P, H], F32, tag="gtmp2", name="t3")
                    nc.vector.tensor_scalar(
                        out=t3, in0=ncv, scalar1=rs2, scalar2=nm2,
                        op0=mybir.AluOpType.mult, op1=mybir.AluOpType.add,
                    )
                    nc.vector.tensor_mul(t3, t3, gc_sb)
                    nc.vector.tensor_add(t3, t3, bc_sb)
                    nc.scalar.activation(tcl, t3, AF.Tanh, bias=0.0, scale=1.0)
                tclns[b] = tcl

            w_tiles = {0: load_w(0)}
            for g in range(4):
                gc0 = g * H
                whi, wlo = w_tiles.pop(g)
                if g + 1 < 4:
                    # prefetch in program order so these DMAs precede this
                    # gate's output stores on the SP queue
                    w_tiles[g + 1] = load_w(g + 1)
                func = AF.Tanh if g == 1 else AF.Sigmoid

                # matmul terms: (lhsT tile, rhs tile, k-pair indices)
                terms = [(xh_hi_sb, whi, range(KS2 // 2))]
                if NA:
                    terms.append((xh_lo_sb, whi, range(NA)))
                if NW:
                    terms.append((xh_hi_sb, wlo, range(NW)))

                # single-block rounds: group closes are spaced one round
                # (~5us) apart, so each block's stats/act chain drains while
                # the next block's matmuls run — no DVE convoy, and the bank
                # rotation (2 banks/round, bufs=8) leaves 4 rounds of slack.
                # Gate 0 front-loads blocks 0-3 in one kp-column phase that
                # tracks the startup DMA arrival order.
                if g == 0:
                    rounds = [range(0, 4)] + [range(i, i + 1) for i in range(4, NB)]
                else:
                    rounds = [range(i, i + 1) for i in range(NB)]

                for rnd, blocks in enumerate(rounds):
                    # per-group matmul schedule; gate-0 round 0 runs
                    # kp-column-major to track the startup DMA arrival order
                    if g == 0 and rnd == 0:
                        sched = [
                            (t, kp)
                            for kp in range(KS2 // 2)
                            for t, (_, _, kps) in enumerate(terms)
                            if kp in kps
                        ]
                    else:
                        sched = [
                            (t, kp)
                            for t, (_, _, kps) in enumerate(terms)
                            for kp in kps
                        ]
                    pss = {}
                    if len(blocks) == 1:
                        mm_order = [
                            (si, t, kp, b, half)
                            for half in range(2)
                            for si, (t, kp) in enumerate(sched)
                            for b in blocks
                        ]
                    else:
                        mm_order = [
                            (si, t, kp, b, half)
                            for si, (t, kp) in enumerate(sched)
                            for b in blocks
                            for half in range(2)
                        ]
                    for si, t, kp, b, half in mm_order:
                        lh, rh, _ = terms[t]
                        b0 = b * P
                        hc = half * 512
                        ps = pss.get((b, half))
                        if ps is None:
                            # per-block PSUM pool (b mod 4): forces bank reuse
                            # distance of 4 blocks regardless of slot policy
                            ps = _PSUMPS[b % 4].tile([P, 512], F32, tag="ps", name="ps")
                            pss[(b, half)] = ps
                        nc.tensor.matmul(
                            ps,
                            lhsT=lh[:, 2 * kp : 2 * kp + 2, b0 : b0 + P],
                            rhs=rh[:, 2 * kp : 2 * kp + 2, hc : hc + 512],
                            start=(si == 0),
                            stop=(trivial and si == len(sched) - 1),
                            perf_mode=DR,
                        )
                    if not trivial:
                        # pre-norm bias, scaled by SCALE^2 on the host
                        for b in blocks:
                            for half in range(2):
                                hc = half * 512
                                nc.tensor.matmul(
                                    pss[(b, half)],
                                    lhsT=ones_t,
                                    rhs=bias_sb[:, gc0 + hc : gc0 + hc + 512],
                                    start=False,
                                    stop=True,
                                )

                    for b in blocks:
                        chain_a(g, b, (pss[(b, 0)], pss[(b, 1)]))
                    # run deferred chains (all but the one just issued):
                    # their rs/nm are ready, so no in-order queue blocks
                    while pendingB:
                        chain_c(*pendingB.pop(0))
                    while len(pendingA) > 1:
                        chain_b(*pendingA.pop(0))

            while pendingA or pendingB:
                while pendingB:
                    chain_c(*pendingB.pop(0))
                if pendingA:
                    chain_b(*pendingA.pop(0))

    _split_fat_waits(nc)
    return nc


_CACHE = {}
LAST_RESULTS = None


def _hi_lo(a32):
    """Split fp32 array into hi+lo e4m3 parts (same scale)."""
    e4 = ml_dtypes.float8_e4m3
    hi = a32.astype(e4)
    lo = (a32 - hi.astype(np.float32)).astype(e4)
    return hi, lo


def kernel(x, c, h, W_xh, W_hh, bias, ln_gamma, ln_beta, ln_c_gamma, ln_c_beta,
           _trace=False):
    x = np.asarray(x, np.float32)
    c = np.asarray(c, np.float32)
    h = np.asarray(h, np.float32)
    W_xh = np.asarray(W_xh, np.float32)
    W_hh = np.asarray(W_hh, np.float32)
    bias = np.asarray(bias, np.float32)
    ln_gamma = np.asarray(ln_gamma, np.float32)
    ln_beta = np.asarray(ln_beta, np.float32)
    ln_c_gamma = np.asarray(ln_c_gamma, np.float32)
    ln_c_beta = np.asarray(ln_c_beta, np.float32)

    trivial = bool(
        (bias == 0).all()
        and (ln_gamma == 1).all()
        and (ln_beta == 0).all()
        and (ln_c_gamma == 1).all()
        and (ln_c_beta == 0).all()
    )

    if trivial not in _CACHE:
        _CACHE[trivial] = _build(trivial)
    nc = _CACHE[trivial]

    # [x h]^T and [W_xh; W_hh], scaled by 2^5 (cancels in the group LN)
    xhT = np.concatenate(
        [np.ascontiguousarray(x.T), np.ascontiguousarray(h.T)], axis=0
    ) * np.float32(SCALE)
    w2 = np.concatenate([W_xh, W_hh], axis=0) * np.float32(SCALE)
    xh_hi, xh_lo = _hi_lo(xhT)
    w_hi, w_lo = _hi_lo(w2)

    in_maps = []
    for i in range(NCORES):
        s = i * BC
        m = {
            "xh_hi": np.ascontiguousarray(xh_hi[:, s : s + BC]),
            "c": np.ascontiguousarray(c[s : s + BC]).astype(ml_dtypes.bfloat16),
            "w_hi": w_hi,
        }
        if NA:
            m["xh_lo"] = np.ascontiguousarray(xh_lo[:, s : s + BC])
        if NW:
            m["w_lo"] = w_lo
        if not trivial:
            m["biasv"] = (bias * np.float32(SCALE * SCALE)).astype(
                ml_dtypes.bfloat16
            ).reshape(1, G4)
            m["g4v"] = ln_gamma.reshape(1, G4)
            m["b4v"] = ln_beta.reshape(1, G4)
            m["gcv"] = ln_c_gamma.reshape(1, H)
            m["bcv"] = ln_c_beta.reshape(1, H)
        in_maps.append(m)

    res = run_bass_kernel_spmd(nc, in_maps, list(range(NCORES)), trace=_trace)
    global LAST_RESULTS
    LAST_RESULTS = res

    out_h = np.concatenate(
        [res.results[i]["new_h"] for i in range(NCORES)], axis=0
    ).astype(np.float32)
    out_c = np.concatenate(
        [res.results[i]["new_c"] for i in range(NCORES)], axis=0
    ).astype(np.float32)
    return out_h, out_c
